# revision 13
# baseline (speedup 1.0000x reference)
"""Trainium2 Bass kernel for nn_DA_conv: per-sample generated depthwise 3x3 conv
-> relu -> 1x1 pointwise conv (+bias) -> + x * channel_attention(altitude).

Data-parallel over batch: 8 samples -> 8 NeuronCores, weights replicated.

Per-core device pipeline:
  prologue:  feat = lrelu(W1 @ alt);  ktab[c,t] = <feat, W2[c*9+t,:]> (9 tiny matmuls)
             att = sigmoid(ca_w2 @ lrelu(ca_w1 @ alt))
             diag_t = diag(ktab[:,t]) built with one DVE tensor_scalar each
  main loop over half-chunks (16 image rows); the 9 depthwise taps run either
  on the TensorEngine (diagonal bf16 matmuls accumulating in PSUM) or, for
  DVE_HALVES, on the VectorEngine (bf16 2x-mode scalar_tensor_tensor chains;
  a one-element-shifted copy xb1 keeps the odd dx taps 4-byte aligned):
    PE  : 9 diagonal matmuls -> psum_s          (PE halves)
    DVE : tensor_scalar + 8 STT -> s_acc bf16   (DVE halves)
    ACT : relu -> srelu (bf16)
    PE  : pointwise conv_w.T @ srelu + diag(att_bf16) @ x_lo into psum_o
    ACT : osb = psum_o + conv_b
    DVE : osb += x_hi * att      (exact fp32 att as the STT scalar)
    DMA : osb -> DRAM  (per 512-column block)

x is sent as a bf16 hi/lo pair (same bytes as fp32, exact sum); the conv taps
use hi only, the residual uses hi (DVE, fp32 att) + lo (PE) so the large
x*att term is nearly exact. Host zero-pads x to [C, 130, 132] (1 row halo,
2+2 column pad) so every tap is a pure access-pattern offset.
"""

import os
from collections import deque
from contextlib import ExitStack

import ml_dtypes
import numpy as np

import concourse.bass as bass
import concourse.mybir as mybir
import concourse.tile as tile
from concourse import bacc
from concourse.bass_utils import run_bass_kernel_spmd

AF = mybir.ActivationFunctionType
ALU = mybir.AluOpType
F32 = mybir.dt.float32
F32R = mybir.dt.float32r
BF16 = mybir.dt.bfloat16

B, C, H, W = 8, 128, 128, 128
KK = 3
NT = KK * KK                 # 9 taps
HW = H * W
XOFF = 2                     # interior column offset in the padded layout
WP = W + 4                   # host-padded width (2 left, 2 right)
HP = H + 2                   # host-padded height (1 halo row each side)
R = 32                       # image rows per chunk
NCH = H // R                 # chunks
BR = 4                       # image rows per psum block (BR*W = 512 fp32 = 1 bank)
NBLK = R // BR               # psum blocks per chunk
HR = 16                      # image rows per half-chunk (scheduling unit)
BPH = HR // BR               # blocks per half-chunk
TAPS = [(dy, dx) for dy in (-1, 0, 1) for dx in (-1, 0, 1)]  # t = (dy+1)*3+(dx+1)
DVE_HALVES = frozenset({1, 3, 6})   # of 8 half-chunks, these run taps on DVE
TAIL_LAG = 2                 # blocks between taps and their consuming tail

last_results = None          # BassKernelResults of the most recent run


def _emit(tc, nc, d):
    ctx = d["ctx"]
    singles = ctx.enter_context(tc.tile_pool(name="singles", bufs=1))
    xpool = ctx.enter_context(tc.tile_pool(name="xpool", bufs=3))
    spool = ctx.enter_context(tc.tile_pool(name="spool", bufs=2))
    apool = ctx.enter_context(tc.tile_pool(name="apool", bufs=2))
    opool = ctx.enter_context(tc.tile_pool(name="opool", bufs=4))
    pss_pool = ctx.enter_context(tc.tile_pool(name="psum_s", bufs=4, space="PSUM"))
    pso_pool = ctx.enter_context(tc.tile_pool(name="psum_o", bufs=3, space="PSUM"))
    pm_pool = ctx.enter_context(tc.tile_pool(name="psum_m", bufs=1, space="PSUM"))

    def load(name, dram, shape, dt=F32):
        t = singles.tile(shape, dt, name=name, tag=name)
        nc.sync.dma_start(out=t, in_=dram)
        return t

    alt = load("alt_s", d["alt"], [128, 1])
    w1t = load("w1t_s", d["w1t"], [128, 128])
    w2t = load("w2t_s", d["w2t"], [128, C * NT])
    cwt = load("cwt_s", d["cwt"], [C, C], dt=BF16)
    cb = load("cb_s", d["cb"], [C, 1])
    ca1t = load("ca1t_s", d["ca1t"], [128, 16])
    ca2t = load("ca2t_s", d["ca2t"], [16, 128])
    iota = load("iota_s", d["iota"], [128, 128])
    cidx = load("cidx_s", d["cidx"], [128, 1])

    def leaky(name, psum_src, parts):
        """lrelu(v) = max(0.1*v, v), via ACT copy to SBUF then one DVE STT."""
        tmp = singles.tile([parts, 1], F32, name=f"{name}_t", tag=f"{name}_t")
        nc.scalar.activation(tmp, psum_src, AF.Copy)
        res = singles.tile([parts, 1], F32, name=name, tag=name)
        nc.vector.scalar_tensor_tensor(
            out=res, in0=tmp, scalar=0.1, in1=tmp, op0=ALU.mult, op1=ALU.max
        )
        return res

    # ---- kernel-generator MLP ----
    feat_ps = pm_pool.tile([128, 1], F32, name="feat_ps", tag="pm")
    nc.tensor.matmul(feat_ps, lhsT=w1t, rhs=alt, start=True, stop=True)
    feat = leaky("feat", feat_ps, 128)

    ktab_ps = pm_pool.tile([128, NT], F32, name="ktab_ps", tag="pm")
    w2r = w2t.rearrange("p (c t) -> p t c", t=NT)
    for t in range(NT):
        nc.tensor.matmul(
            ktab_ps[:, t : t + 1], lhsT=w2r[:, t, :], rhs=feat, start=True, stop=True
        )
    ktab = singles.tile([128, NT], F32, name="ktab", tag="ktab")
    nc.scalar.activation(ktab, ktab_ps, AF.Copy)

    # ---- channel attention ----
    a1_ps = pm_pool.tile([16, 1], F32, name="a1_ps", tag="pm")
    nc.tensor.matmul(a1_ps, lhsT=ca1t, rhs=alt, start=True, stop=True)
    a1 = leaky("a1", a1_ps, 16)
    att_ps = pm_pool.tile([128, 1], F32, name="att_ps", tag="pm")
    nc.tensor.matmul(att_ps, lhsT=ca2t, rhs=a1, start=True, stop=True)
    attv = singles.tile([128, 1], F32, name="attv", tag="attv")
    nc.scalar.activation(attv, att_ps, AF.Sigmoid)

    # ---- diagonal weight matrices ----
    diags = []
    for t in range(NT):
        dg = singles.tile([128, 128], BF16, name=f"diag{t}", tag=f"diag{t}")
        nc.vector.tensor_scalar(
            out=dg, in0=iota, scalar1=cidx, scalar2=ktab[:, t : t + 1],
            op0=ALU.is_equal, op1=ALU.mult,
        )
        diags.append(dg)
    attd = singles.tile([128, 128], BF16, name="attd", tag="attd")
    nc.vector.tensor_scalar(
        out=attd, in0=iota, scalar1=cidx, scalar2=attv,
        op0=ALU.is_equal, op1=ALU.mult,
    )

    x3h = d["xpad_hi"].rearrange("c (h w) -> c h w", w=WP)
    x3l = d["xpad_lo"].rearrange("c (h w) -> c h w", w=WP)
    out_d = d["out"]

    # ---- main loop over half-chunks, tails pipelined TAIL_LAG blocks late ----
    tails = deque()

    def flush(n):
        while len(tails) > n:
            tails.popleft()()

    for ci in range(NCH):
        y0 = ci * R
        xp = xpool.tile([128, R + 2, WP], BF16, name=f"xp{ci}", tag="xp")
        nc.sync.dma_start(out=xp, in_=x3h[:, y0 : y0 + R + 2, :])
        xpl = xpool.tile([128, R + 2, WP], BF16, name=f"xpl{ci}", tag="xpl")
        nc.sync.dma_start(out=xpl, in_=x3l[:, y0 : y0 + R + 2, :])
        xb1 = None
        if any((2 * ci + hh) in DVE_HALVES for hh in (0, 1)):
            # xb1[n] = xp_flat[n+1]: keeps dx=+-1 taps 4-byte aligned on DVE
            nflat = (R + 2) * WP
            xb1 = xpool.tile([128, nflat], BF16, name=f"xb1{ci}", tag="xb1")
            nc.vector.tensor_copy(
                out=xb1[:, 0 : nflat - 2],
                in_=xp.rearrange("p r c -> p (r c)")[:, 1 : nflat - 1],
            )
        srelu = spool.tile([128, R * W], BF16, name=f"sr{ci}", tag="sr")

        for h in (0, 1):
            u = 2 * ci + h
            hr0 = h * HR  # chunk-relative first image row of this half
            if u in DVE_HALVES:
                xb13 = xb1.rearrange("p (r c) -> p r c", c=WP)
                sacc = apool.tile([128, HR * W], BF16, name=f"sacc{u}", tag="sacc")
                for ti, (dy, dx) in enumerate(TAPS):
                    if dx == 0:
                        src = xp[:, 1 + hr0 + dy : 1 + hr0 + dy + HR, XOFF : XOFF + W]
                    elif dx == 1:
                        src = xb13[:, 1 + hr0 + dy : 1 + hr0 + dy + HR, XOFF : XOFF + W]
                    else:
                        src = xb13[:, 1 + hr0 + dy : 1 + hr0 + dy + HR, 0:W]
                    if ti == 0:
                        nc.vector.tensor_scalar_mul(
                            out=sacc, in0=src, scalar1=ktab[:, ti : ti + 1]
                        )
                    else:
                        nc.vector.scalar_tensor_tensor(
                            out=sacc, in0=src, scalar=ktab[:, ti : ti + 1],
                            in1=sacc, op0=ALU.mult, op1=ALU.add,
                        )
                sl_h = slice(hr0 * W, (hr0 + HR) * W)
                nc.scalar.activation(srelu[:, sl_h], sacc, AF.Relu)
                for bb in range(BPH):
                    r0 = hr0 + bb * BR
                    tails.append(_make_tail(nc, pso_pool, opool, xp, xpl, srelu,
                                            None, cwt, attd, attv, cb, out_d,
                                            ci, r0, y0))
                    flush(TAIL_LAG)
            else:
                for bb in range(BPH):
                    r0 = hr0 + bb * BR
                    pss = pss_pool.tile([128, BR * W], F32, name=f"pss{u}_{bb}",
                                        tag="pss")
                    for ti, (dy, dx) in enumerate(TAPS):
                        rhs = xp[:, 1 + r0 + dy : 1 + r0 + dy + BR,
                                 XOFF + dx : XOFF + dx + W]
                        nc.tensor.matmul(
                            pss, lhsT=diags[ti], rhs=rhs,
                            start=(ti == 0), stop=(ti == NT - 1),
                        )
                    tails.append(_make_tail(nc, pso_pool, opool, xp, xpl, srelu,
                                            pss, cwt, attd, attv, cb, out_d,
                                            ci, r0, y0))
                    flush(TAIL_LAG)
    flush(0)


def _make_tail(nc, pso_pool, opool, xp, xpl, srelu, pss, cwt, attd, attv, cb,
               out_d, ci, r0, y0):
    """relu (PE halves) + pointwise + lo-residual + biased evac + hi-residual +
    store for the block at chunk-relative rows [r0, r0+BR)."""

    def tail():
        sl = slice(r0 * W, (r0 + BR) * W)
        if pss is not None:
            nc.scalar.activation(srelu[:, sl], pss, AF.Relu)
        pso = pso_pool.tile([128, BR * W], F32, name=f"pso{ci}_{r0}", tag="pso")
        nc.tensor.matmul(pso, lhsT=cwt, rhs=srelu[:, sl], start=True, stop=False)
        nc.tensor.matmul(
            pso, lhsT=attd, rhs=xpl[:, 1 + r0 : 1 + r0 + BR, XOFF : XOFF + W],
            start=False, stop=True,
        )
        osb = opool.tile([128, BR * W], F32, name=f"ob{ci}_{r0}", tag="ob")
        nc.scalar.activation(osb, pso, AF.Identity, bias=cb)
        nc.vector.scalar_tensor_tensor(
            out=osb, in0=xp[:, 1 + r0 : 1 + r0 + BR, XOFF : XOFF + W],
            scalar=attv, in1=osb, op0=ALU.mult, op1=ALU.add,
        )
        nc.sync.dma_start(out=out_d[:, (y0 + r0) * W : (y0 + r0 + BR) * W], in_=osb)

    return tail


def build_module():
    nc = bacc.Bacc(
        "TRN2",
        target_bir_lowering=False,
        debug=False,
        enable_asserts=False,
        num_devices=B,
    )
    d = {
        "xpad_hi": nc.dram_tensor("xpad_hi", [C, HP * WP], BF16, kind="ExternalInput").ap(),
        "xpad_lo": nc.dram_tensor("xpad_lo", [C, HP * WP], BF16, kind="ExternalInput").ap(),
        "alt": nc.dram_tensor("alt", [128, 1], F32, kind="ExternalInput").ap(),
        "w1t": nc.dram_tensor("w1t", [128, 128], F32, kind="ExternalInput").ap(),
        "w2t": nc.dram_tensor("w2t", [128, C * NT], F32, kind="ExternalInput").ap(),
        "cwt": nc.dram_tensor("cwt", [C, C], BF16, kind="ExternalInput").ap(),
        "cb": nc.dram_tensor("cb", [C, 1], F32, kind="ExternalInput").ap(),
        "ca1t": nc.dram_tensor("ca1t", [128, 16], F32, kind="ExternalInput").ap(),
        "ca2t": nc.dram_tensor("ca2t", [16, 128], F32, kind="ExternalInput").ap(),
        "iota": nc.dram_tensor("iota", [128, 128], F32, kind="ExternalInput").ap(),
        "cidx": nc.dram_tensor("cidx", [128, 1], F32, kind="ExternalInput").ap(),
        "out": nc.dram_tensor("out", [C, HW], F32, kind="ExternalOutput").ap(),
    }
    with tile.TileContext(nc) as tc:
        with ExitStack() as ctx:
            d["ctx"] = ctx
            _emit(tc, nc, d)
    nc.finalize()
    return nc


_module_cache = None


def _get_module():
    global _module_cache
    if _module_cache is None:
        _module_cache = build_module()
    return _module_cache


def make_in_maps(x, altitude, W1, W2, conv_w, conv_b, ca_w1, ca_w2):
    f = np.float32
    x = np.asarray(x, dtype=f)
    altitude = np.asarray(altitude, dtype=f)
    xpad = np.zeros((B, C, HP, WP), dtype=f)
    xpad[:, :, 1 : H + 1, XOFF : XOFF + W] = x
    xhi_f = xpad.astype(ml_dtypes.bfloat16)
    xlo = np.ascontiguousarray(
        (xpad - xhi_f.astype(f)).astype(ml_dtypes.bfloat16).reshape(B, C, HP * WP)
    )
    xhi = np.ascontiguousarray(xhi_f.reshape(B, C, HP * WP))
    shared = {
        "w1t": np.ascontiguousarray(np.asarray(W1, dtype=f).T),
        "w2t": np.ascontiguousarray(np.asarray(W2, dtype=f).T),
        "cwt": np.ascontiguousarray(
            np.asarray(conv_w, dtype=f).T.astype(ml_dtypes.bfloat16)
        ),
        "cb": np.ascontiguousarray(np.asarray(conv_b, dtype=f).reshape(C, 1)),
        "ca1t": np.ascontiguousarray(np.asarray(ca_w1, dtype=f).T),
        "ca2t": np.ascontiguousarray(np.asarray(ca_w2, dtype=f).T),
        "iota": np.ascontiguousarray(
            np.broadcast_to(np.arange(128, dtype=f), (128, 128))
        ),
        "cidx": np.arange(128, dtype=f).reshape(128, 1).copy(),
    }
    return [
        dict(shared, xpad_hi=xhi[bb], xpad_lo=xlo[bb],
             alt=np.ascontiguousarray(altitude[bb].reshape(128, 1)))
        for bb in range(B)
    ]


def kernel(x, altitude, W1, W2, conv_w, conv_b, ca_w1, ca_w2):
    global last_results
    in_maps = make_in_maps(x, altitude, W1, W2, conv_w, conv_b, ca_w1, ca_w2)
    nc = _get_module()
    trace = os.environ.get("KERNEL_TRACE", "0") == "1"
    last_results = run_bass_kernel_spmd(
        nc, in_maps, core_ids=list(range(B)), trace=trace
    )
    out = np.stack(
        [last_results.results[bb]["out"].reshape(C, H, W) for bb in range(B)]
    )
    return out


# revision 18
# speedup vs baseline: 1.0809x; 1.0809x over previous
"""Trainium2 Bass kernel for nn_DA_conv: per-sample generated depthwise 3x3 conv
-> relu -> 1x1 pointwise conv (+bias) -> + x * channel_attention(altitude).

Data-parallel over batch: 8 samples -> 8 NeuronCores, weights replicated.

Per-core device pipeline:
  prologue:  feat = lrelu(W1 @ alt);  ktab[c,t] = <feat, W2[c*9+t,:]> (9 tiny matmuls)
             att = sigmoid(ca_w2 @ lrelu(ca_w1 @ alt))
             diag_t = diag(ktab[:,t]) built with one DVE tensor_scalar each
  main loop over half-chunks (16 image rows); the 9 depthwise taps run either
  on the TensorEngine (diagonal bf16 matmuls accumulating in PSUM) or, for
  DVE_HALVES, on the VectorEngine (bf16 2x-mode scalar_tensor_tensor chains;
  a one-element-shifted copy xb1 keeps the odd dx taps 4-byte aligned):
    PE  : 9 diagonal matmuls -> psum_s          (PE halves)
    DVE : tensor_scalar + 8 STT -> s_acc bf16   (DVE halves)
    ACT : relu -> srelu (bf16)
    PE  : pointwise conv_w.T @ srelu + diag(att_bf16) @ x_lo into psum_o
    ACT : osb = psum_o + conv_b
    DVE : osb += x_hi * att      (exact fp32 att as the STT scalar)
    DMA : osb -> DRAM  (per 512-column block)

x is sent as a bf16 hi/lo pair (same bytes as fp32, exact sum); the conv taps
use hi only, the residual uses hi (DVE, fp32 att) + lo (PE) so the large
x*att term is nearly exact. Host zero-pads x to [C, 130, 132] (1 row halo,
2+2 column pad) so every tap is a pure access-pattern offset.
"""

import os
from collections import deque
from contextlib import ExitStack

import ml_dtypes
import numpy as np

import concourse.bass as bass
import concourse.mybir as mybir
import concourse.tile as tile
from concourse import bacc
from concourse.bass_utils import run_bass_kernel_spmd

AF = mybir.ActivationFunctionType
ALU = mybir.AluOpType
F32 = mybir.dt.float32
F32R = mybir.dt.float32r
BF16 = mybir.dt.bfloat16

B, C, H, W = 8, 128, 128, 128
KK = 3
NT = KK * KK                 # 9 taps
HW = H * W
XOFF = 2                     # interior column offset in the padded layout
WP = W + 4                   # host-padded width (2 left, 2 right)
HP = H + 2                   # host-padded height (1 halo row each side)
R = 32                       # image rows per chunk
NCH = H // R                 # chunks
BR = 4                       # image rows per psum block (BR*W = 512 fp32 = 1 bank)
NBLK = R // BR               # psum blocks per chunk
HR = 16                      # image rows per half-chunk (scheduling unit)
BPH = HR // BR               # blocks per half-chunk
TAPS = [(dy, dx) for dy in (-1, 0, 1) for dx in (-1, 0, 1)]  # t = (dy+1)*3+(dx+1)
DVE_HALVES = frozenset()   # DVE tap path disabled (STT runs 1x-only: net loss)
TAIL_LAG = 2                 # blocks between taps and their consuming tail

last_results = None          # BassKernelResults of the most recent run


def _emit(tc, nc, d):
    ctx = d["ctx"]
    singles = ctx.enter_context(tc.tile_pool(name="singles", bufs=1))
    xpool = ctx.enter_context(tc.tile_pool(name="xpool", bufs=3))
    spool = ctx.enter_context(tc.tile_pool(name="spool", bufs=2))
    apool = ctx.enter_context(tc.tile_pool(name="apool", bufs=2))
    opool = ctx.enter_context(tc.tile_pool(name="opool", bufs=4))
    pss_pool = ctx.enter_context(tc.tile_pool(name="psum_s", bufs=4, space="PSUM"))
    pso_pool = ctx.enter_context(tc.tile_pool(name="psum_o", bufs=3, space="PSUM"))
    pm_pool = ctx.enter_context(tc.tile_pool(name="psum_m", bufs=1, space="PSUM"))

    def load(name, dram, shape, dt=F32):
        t = singles.tile(shape, dt, name=name, tag=name)
        nc.sync.dma_start(out=t, in_=dram)
        return t

    alt = load("alt_s", d["alt"], [128, 1])
    w1t = load("w1t_s", d["w1t"], [128, 128])
    w2t = load("w2t_s", d["w2t"], [128, C * NT])
    cwt = load("cwt_s", d["cwt"], [C, C], dt=BF16)
    cb = load("cb_s", d["cb"], [C, 1])
    ca1t = load("ca1t_s", d["ca1t"], [128, 16])
    ca2t = load("ca2t_s", d["ca2t"], [16, 128])
    iota = load("iota_s", d["iota"], [128, 128])
    cidx = load("cidx_s", d["cidx"], [128, 1])

    def leaky(name, psum_src, parts):
        """lrelu(v) = max(0.1*v, v), via ACT copy to SBUF then one DVE STT."""
        tmp = singles.tile([parts, 1], F32, name=f"{name}_t", tag=f"{name}_t")
        nc.scalar.activation(tmp, psum_src, AF.Copy)
        res = singles.tile([parts, 1], F32, name=name, tag=name)
        nc.vector.scalar_tensor_tensor(
            out=res, in0=tmp, scalar=0.1, in1=tmp, op0=ALU.mult, op1=ALU.max
        )
        return res

    # ---- kernel-generator MLP ----
    feat_ps = pm_pool.tile([128, 1], F32, name="feat_ps", tag="pm")
    nc.tensor.matmul(feat_ps, lhsT=w1t, rhs=alt, start=True, stop=True)
    feat = leaky("feat", feat_ps, 128)

    ktab_ps = pm_pool.tile([128, NT], F32, name="ktab_ps", tag="pm")
    w2r = w2t.rearrange("p (c t) -> p t c", t=NT)
    for t in range(NT):
        nc.tensor.matmul(
            ktab_ps[:, t : t + 1], lhsT=w2r[:, t, :], rhs=feat, start=True, stop=True
        )
    ktab = singles.tile([128, NT], F32, name="ktab", tag="ktab")
    nc.scalar.activation(ktab, ktab_ps, AF.Copy)

    # ---- channel attention ----
    a1_ps = pm_pool.tile([16, 1], F32, name="a1_ps", tag="pm")
    nc.tensor.matmul(a1_ps, lhsT=ca1t, rhs=alt, start=True, stop=True)
    a1 = leaky("a1", a1_ps, 16)
    att_ps = pm_pool.tile([128, 1], F32, name="att_ps", tag="pm")
    nc.tensor.matmul(att_ps, lhsT=ca2t, rhs=a1, start=True, stop=True)
    attv = singles.tile([128, 1], F32, name="attv", tag="attv")
    nc.scalar.activation(attv, att_ps, AF.Sigmoid)

    # ---- diagonal weight matrices ----
    diags = []
    for t in range(NT):
        dg = singles.tile([128, 128], BF16, name=f"diag{t}", tag=f"diag{t}")
        nc.vector.tensor_scalar(
            out=dg, in0=iota, scalar1=cidx, scalar2=ktab[:, t : t + 1],
            op0=ALU.is_equal, op1=ALU.mult,
        )
        diags.append(dg)
    attd = singles.tile([128, 128], BF16, name="attd", tag="attd")
    nc.vector.tensor_scalar(
        out=attd, in0=iota, scalar1=cidx, scalar2=attv,
        op0=ALU.is_equal, op1=ALU.mult,
    )

    x3h = d["xpad_hi"].rearrange("c (h w) -> c h w", w=WP)
    x3l = d["xpad_lo"].rearrange("c (h w) -> c h w", w=WP)
    out_d = d["out"]

    # ---- main loop over half-chunks, tails pipelined TAIL_LAG blocks late ----
    tails = deque()

    def flush(n):
        while len(tails) > n:
            tails.popleft()()

    for ci in range(NCH):
        y0 = ci * R
        xp = xpool.tile([128, R + 2, WP], BF16, name=f"xp{ci}", tag="xp")
        nc.sync.dma_start(out=xp, in_=x3h[:, y0 : y0 + R + 2, :])
        xpl = xpool.tile([128, R + 2, WP], BF16, name=f"xpl{ci}", tag="xpl")
        nc.sync.dma_start(out=xpl, in_=x3l[:, y0 : y0 + R + 2, :])
        xb1 = None
        if any((2 * ci + hh) in DVE_HALVES for hh in (0, 1)):
            # xb1[n] = xp_flat[n+1]: keeps dx=+-1 taps 4-byte aligned on DVE
            nflat = (R + 2) * WP
            xb1 = xpool.tile([128, nflat], BF16, name=f"xb1{ci}", tag="xb1")
            nc.vector.tensor_copy(
                out=xb1[:, 0 : nflat - 2],
                in_=xp.rearrange("p r c -> p (r c)")[:, 1 : nflat - 1],
            )
        srelu = spool.tile([128, R * W], BF16, name=f"sr{ci}", tag="sr")

        for h in (0, 1):
            u = 2 * ci + h
            hr0 = h * HR  # chunk-relative first image row of this half
            if u in DVE_HALVES:
                xb13 = xb1.rearrange("p (r c) -> p r c", c=WP)
                sacc = apool.tile([128, HR * W], BF16, name=f"sacc{u}", tag="sacc")
                for ti, (dy, dx) in enumerate(TAPS):
                    if dx == 0:
                        src = xp[:, 1 + hr0 + dy : 1 + hr0 + dy + HR, XOFF : XOFF + W]
                    elif dx == 1:
                        src = xb13[:, 1 + hr0 + dy : 1 + hr0 + dy + HR, XOFF : XOFF + W]
                    else:
                        src = xb13[:, 1 + hr0 + dy : 1 + hr0 + dy + HR, 0:W]
                    if ti == 0:
                        nc.vector.tensor_scalar_mul(
                            out=sacc, in0=src, scalar1=ktab[:, ti : ti + 1]
                        )
                    else:
                        nc.vector.scalar_tensor_tensor(
                            out=sacc, in0=src, scalar=ktab[:, ti : ti + 1],
                            in1=sacc, op0=ALU.mult, op1=ALU.add,
                        )
                sl_h = slice(hr0 * W, (hr0 + HR) * W)
                nc.scalar.activation(srelu[:, sl_h], sacc, AF.Relu)
                for bb in range(BPH):
                    r0 = hr0 + bb * BR
                    tails.append(_make_tail(nc, pso_pool, opool, xp, xpl, srelu,
                                            None, cwt, attd, attv, cb, out_d,
                                            ci, r0, y0))
                    flush(TAIL_LAG)
            else:
                for bb in range(BPH):
                    r0 = hr0 + bb * BR
                    pss = pss_pool.tile([128, BR * W], F32, name=f"pss{u}_{bb}",
                                        tag="pss")
                    for ti, (dy, dx) in enumerate(TAPS):
                        rhs = xp[:, 1 + r0 + dy : 1 + r0 + dy + BR,
                                 XOFF + dx : XOFF + dx + W]
                        nc.tensor.matmul(
                            pss, lhsT=diags[ti], rhs=rhs,
                            start=(ti == 0), stop=(ti == NT - 1),
                        )
                    tails.append(_make_tail(nc, pso_pool, opool, xp, xpl, srelu,
                                            pss, cwt, attd, attv, cb, out_d,
                                            ci, r0, y0))
                    flush(TAIL_LAG)
    flush(0)


def _make_tail(nc, pso_pool, opool, xp, xpl, srelu, pss, cwt, attd, attv, cb,
               out_d, ci, r0, y0):
    """relu (PE halves) + pointwise + lo-residual + biased evac + hi-residual +
    store for the block at chunk-relative rows [r0, r0+BR)."""

    def tail():
        sl = slice(r0 * W, (r0 + BR) * W)
        if pss is not None:
            nc.scalar.activation(srelu[:, sl], pss, AF.Relu)
        pso = pso_pool.tile([128, BR * W], F32, name=f"pso{ci}_{r0}", tag="pso")
        nc.tensor.matmul(pso, lhsT=cwt, rhs=srelu[:, sl], start=True, stop=False)
        nc.tensor.matmul(
            pso, lhsT=attd, rhs=xpl[:, 1 + r0 : 1 + r0 + BR, XOFF : XOFF + W],
            start=False, stop=True,
        )
        osb = opool.tile([128, BR * W], F32, name=f"ob{ci}_{r0}", tag="ob")
        nc.scalar.activation(osb, pso, AF.Identity, bias=cb)
        nc.vector.scalar_tensor_tensor(
            out=osb, in0=xp[:, 1 + r0 : 1 + r0 + BR, XOFF : XOFF + W],
            scalar=attv, in1=osb, op0=ALU.mult, op1=ALU.add,
        )
        nc.sync.dma_start(out=out_d[:, (y0 + r0) * W : (y0 + r0 + BR) * W], in_=osb)

    return tail


def build_module():
    nc = bacc.Bacc(
        "TRN2",
        target_bir_lowering=False,
        debug=False,
        enable_asserts=False,
        num_devices=B,
    )
    d = {
        "xpad_hi": nc.dram_tensor("xpad_hi", [C, HP * WP], BF16, kind="ExternalInput").ap(),
        "xpad_lo": nc.dram_tensor("xpad_lo", [C, HP * WP], BF16, kind="ExternalInput").ap(),
        "alt": nc.dram_tensor("alt", [128, 1], F32, kind="ExternalInput").ap(),
        "w1t": nc.dram_tensor("w1t", [128, 128], F32, kind="ExternalInput").ap(),
        "w2t": nc.dram_tensor("w2t", [128, C * NT], F32, kind="ExternalInput").ap(),
        "cwt": nc.dram_tensor("cwt", [C, C], BF16, kind="ExternalInput").ap(),
        "cb": nc.dram_tensor("cb", [C, 1], F32, kind="ExternalInput").ap(),
        "ca1t": nc.dram_tensor("ca1t", [128, 16], F32, kind="ExternalInput").ap(),
        "ca2t": nc.dram_tensor("ca2t", [16, 128], F32, kind="ExternalInput").ap(),
        "iota": nc.dram_tensor("iota", [128, 128], F32, kind="ExternalInput").ap(),
        "cidx": nc.dram_tensor("cidx", [128, 1], F32, kind="ExternalInput").ap(),
        "out": nc.dram_tensor("out", [C, HW], F32, kind="ExternalOutput").ap(),
    }
    with tile.TileContext(nc) as tc:
        with ExitStack() as ctx:
            d["ctx"] = ctx
            _emit(tc, nc, d)
    nc.finalize()
    return nc


_module_cache = None


def _get_module():
    global _module_cache
    if _module_cache is None:
        _module_cache = build_module()
    return _module_cache


def make_in_maps(x, altitude, W1, W2, conv_w, conv_b, ca_w1, ca_w2):
    f = np.float32
    x = np.asarray(x, dtype=f)
    altitude = np.asarray(altitude, dtype=f)
    xpad = np.zeros((B, C, HP, WP), dtype=f)
    xpad[:, :, 1 : H + 1, XOFF : XOFF + W] = x
    xhi_f = xpad.astype(ml_dtypes.bfloat16)
    xlo = np.ascontiguousarray(
        (xpad - xhi_f.astype(f)).astype(ml_dtypes.bfloat16).reshape(B, C, HP * WP)
    )
    xhi = np.ascontiguousarray(xhi_f.reshape(B, C, HP * WP))
    shared = {
        "w1t": np.ascontiguousarray(np.asarray(W1, dtype=f).T),
        "w2t": np.ascontiguousarray(np.asarray(W2, dtype=f).T),
        "cwt": np.ascontiguousarray(
            np.asarray(conv_w, dtype=f).T.astype(ml_dtypes.bfloat16)
        ),
        "cb": np.ascontiguousarray(np.asarray(conv_b, dtype=f).reshape(C, 1)),
        "ca1t": np.ascontiguousarray(np.asarray(ca_w1, dtype=f).T),
        "ca2t": np.ascontiguousarray(np.asarray(ca_w2, dtype=f).T),
        "iota": np.ascontiguousarray(
            np.broadcast_to(np.arange(128, dtype=f), (128, 128))
        ),
        "cidx": np.arange(128, dtype=f).reshape(128, 1).copy(),
    }
    return [
        dict(shared, xpad_hi=xhi[bb], xpad_lo=xlo[bb],
             alt=np.ascontiguousarray(altitude[bb].reshape(128, 1)))
        for bb in range(B)
    ]


def kernel(x, altitude, W1, W2, conv_w, conv_b, ca_w1, ca_w2):
    global last_results
    in_maps = make_in_maps(x, altitude, W1, W2, conv_w, conv_b, ca_w1, ca_w2)
    nc = _get_module()
    trace = os.environ.get("KERNEL_TRACE", "0") == "1"
    last_results = run_bass_kernel_spmd(
        nc, in_maps, core_ids=list(range(B)), trace=trace
    )
    out = np.stack(
        [last_results.results[bb]["out"].reshape(C, H, W) for bb in range(B)]
    )
    return out


# revision 19
# speedup vs baseline: 1.1271x; 1.0428x over previous
"""Trainium2 Bass kernel for nn_DA_conv: per-sample generated depthwise 3x3 conv
-> relu -> 1x1 pointwise conv (+bias) -> + x * channel_attention(altitude).

Data-parallel over batch: 8 samples -> 8 NeuronCores, weights replicated.

Per-core device pipeline:
  prologue:  feat = lrelu(W1 @ alt);  ktab[c,t] = <feat, W2[c*9+t,:]> (9 tiny matmuls)
             att = sigmoid(ca_w2 @ lrelu(ca_w1 @ alt))
             diag_t = diag(ktab[:,t]) built with one DVE tensor_scalar each
  main loop over half-chunks (16 image rows); the 9 depthwise taps run either
  on the TensorEngine (diagonal bf16 matmuls accumulating in PSUM) or, for
  DVE_HALVES, on the VectorEngine (bf16 2x-mode scalar_tensor_tensor chains;
  a one-element-shifted copy xb1 keeps the odd dx taps 4-byte aligned):
    PE  : 9 diagonal matmuls -> psum_s          (PE halves)
    DVE : tensor_scalar + 8 STT -> s_acc bf16   (DVE halves)
    ACT : relu -> srelu (bf16)
    PE  : pointwise conv_w.T @ srelu + diag(att_bf16) @ x_lo into psum_o
    ACT : osb = psum_o + conv_b
    DVE : osb += x_hi * att      (exact fp32 att as the STT scalar)
    DMA : osb -> DRAM  (per 512-column block)

x is sent as a bf16 hi/lo pair (same bytes as fp32, exact sum); the conv taps
use hi only, the residual uses hi (DVE, fp32 att) + lo (PE) so the large
x*att term is nearly exact. Host zero-pads x to [C, 130, 132] (1 row halo,
2+2 column pad) so every tap is a pure access-pattern offset.
"""

import os
from collections import deque
from contextlib import ExitStack

import ml_dtypes
import numpy as np

import concourse.bass as bass
import concourse.mybir as mybir
import concourse.tile as tile
from concourse import bacc
from concourse.bass_utils import run_bass_kernel_spmd

AF = mybir.ActivationFunctionType
ALU = mybir.AluOpType
F32 = mybir.dt.float32
F32R = mybir.dt.float32r
BF16 = mybir.dt.bfloat16

B, C, H, W = 8, 128, 128, 128
KK = 3
NT = KK * KK                 # 9 taps
HW = H * W
XOFF = 2                     # interior column offset in the padded layout
WP = W + 4                   # host-padded width (2 left, 2 right)
HP = H + 2                   # host-padded height (1 halo row each side)
R = 32                       # image rows per chunk
NCH = H // R                 # chunks
BR = 4                       # image rows per psum block (BR*W = 512 fp32 = 1 bank)
NBLK = R // BR               # psum blocks per chunk
HR = 16                      # image rows per half-chunk (scheduling unit)
BPH = HR // BR               # blocks per half-chunk
TAPS = [(dy, dx) for dy in (-1, 0, 1) for dx in (-1, 0, 1)]  # t = (dy+1)*3+(dx+1)
DVE_HALVES = frozenset()   # DVE tap path disabled (STT runs 1x-only: net loss)
TAIL_LAG = 2                 # blocks between taps and their consuming tail

last_results = None          # BassKernelResults of the most recent run


def _emit(tc, nc, d):
    ctx = d["ctx"]
    singles = ctx.enter_context(tc.tile_pool(name="singles", bufs=1))
    xpool = ctx.enter_context(tc.tile_pool(name="xpool", bufs=3))
    spool = ctx.enter_context(tc.tile_pool(name="spool", bufs=2))
    apool = ctx.enter_context(tc.tile_pool(name="apool", bufs=2))
    opool = ctx.enter_context(tc.tile_pool(name="opool", bufs=4))
    pss_pool = ctx.enter_context(tc.tile_pool(name="psum_s", bufs=4, space="PSUM"))
    pso_pool = ctx.enter_context(tc.tile_pool(name="psum_o", bufs=3, space="PSUM"))
    pm_pool = ctx.enter_context(tc.tile_pool(name="psum_m", bufs=1, space="PSUM"))

    def load(name, dram, shape, dt=F32):
        t = singles.tile(shape, dt, name=name, tag=name)
        nc.sync.dma_start(out=t, in_=dram)
        return t

    alt = load("alt_s", d["alt"], [128, 1])
    w1t = load("w1t_s", d["w1t"], [128, 128])
    w2t = load("w2t_s", d["w2t"], [128, C * NT])
    cwt = load("cwt_s", d["cwt"], [C, C], dt=BF16)
    cb = load("cb_s", d["cb"], [C, 1])
    ca1t = load("ca1t_s", d["ca1t"], [128, 16])
    ca2t = load("ca2t_s", d["ca2t"], [16, 128])
    iota = load("iota_s", d["iota"], [128, 128])
    cidx = load("cidx_s", d["cidx"], [128, 1])

    def leaky(name, psum_src, parts):
        """lrelu(v) = max(0.1*v, v), via ACT copy to SBUF then one DVE STT."""
        tmp = singles.tile([parts, 1], F32, name=f"{name}_t", tag=f"{name}_t")
        nc.scalar.activation(tmp, psum_src, AF.Copy)
        res = singles.tile([parts, 1], F32, name=name, tag=name)
        nc.vector.scalar_tensor_tensor(
            out=res, in0=tmp, scalar=0.1, in1=tmp, op0=ALU.mult, op1=ALU.max
        )
        return res

    # ---- kernel-generator MLP ----
    feat_ps = pm_pool.tile([128, 1], F32, name="feat_ps", tag="pm")
    nc.tensor.matmul(feat_ps, lhsT=w1t, rhs=alt, start=True, stop=True)
    feat = leaky("feat", feat_ps, 128)

    ktab_ps = pm_pool.tile([128, NT], F32, name="ktab_ps", tag="pm")
    w2r = w2t.rearrange("p (c t) -> p t c", t=NT)
    for t in range(NT):
        nc.tensor.matmul(
            ktab_ps[:, t : t + 1], lhsT=w2r[:, t, :], rhs=feat, start=True, stop=True
        )
    ktab = singles.tile([128, NT], F32, name="ktab", tag="ktab")
    nc.scalar.activation(ktab, ktab_ps, AF.Copy)

    # ---- channel attention ----
    a1_ps = pm_pool.tile([16, 1], F32, name="a1_ps", tag="pm")
    nc.tensor.matmul(a1_ps, lhsT=ca1t, rhs=alt, start=True, stop=True)
    a1 = leaky("a1", a1_ps, 16)
    att_ps = pm_pool.tile([128, 1], F32, name="att_ps", tag="pm")
    nc.tensor.matmul(att_ps, lhsT=ca2t, rhs=a1, start=True, stop=True)
    attv = singles.tile([128, 1], F32, name="attv", tag="attv")
    nc.scalar.activation(attv, att_ps, AF.Sigmoid)

    # ---- diagonal weight matrices ----
    diags = []
    for t in range(NT):
        dg = singles.tile([128, 128], BF16, name=f"diag{t}", tag=f"diag{t}")
        nc.vector.tensor_scalar(
            out=dg, in0=iota, scalar1=cidx, scalar2=ktab[:, t : t + 1],
            op0=ALU.is_equal, op1=ALU.mult,
        )
        diags.append(dg)
    attd = singles.tile([128, 128], BF16, name="attd", tag="attd")
    nc.vector.tensor_scalar(
        out=attd, in0=iota, scalar1=cidx, scalar2=attv,
        op0=ALU.is_equal, op1=ALU.mult,
    )

    x3h = d["xpad_hi"].rearrange("c (h w) -> c h w", w=WP)
    x3l = d["xpad_lo"].rearrange("c (h w) -> c h w", w=WP)
    out_d = d["out"]

    # ---- main loop over half-chunks, tails pipelined TAIL_LAG blocks late ----
    tails = deque()

    def flush(n):
        while len(tails) > n:
            tails.popleft()()

    for ci in range(NCH):
        y0 = ci * R
        xp = xpool.tile([128, R + 2, WP], BF16, name=f"xp{ci}", tag="xp")
        nc.sync.dma_start(out=xp, in_=x3h[:, y0 : y0 + R + 2, :])
        xpl = xpool.tile([128, R + 2, WP], BF16, name=f"xpl{ci}", tag="xpl")
        nc.sync.dma_start(out=xpl, in_=x3l[:, y0 : y0 + R + 2, :])
        xb1 = None
        if any((2 * ci + hh) in DVE_HALVES for hh in (0, 1)):
            # xb1[n] = xp_flat[n+1]: keeps dx=+-1 taps 4-byte aligned on DVE
            nflat = (R + 2) * WP
            xb1 = xpool.tile([128, nflat], BF16, name=f"xb1{ci}", tag="xb1")
            nc.vector.tensor_copy(
                out=xb1[:, 0 : nflat - 2],
                in_=xp.rearrange("p r c -> p (r c)")[:, 1 : nflat - 1],
            )
        srelu = spool.tile([128, R * W], BF16, name=f"sr{ci}", tag="sr")

        for h in (0, 1):
            u = 2 * ci + h
            hr0 = h * HR  # chunk-relative first image row of this half
            if u in DVE_HALVES:
                xb13 = xb1.rearrange("p (r c) -> p r c", c=WP)
                sacc = apool.tile([128, HR * W], BF16, name=f"sacc{u}", tag="sacc")
                for ti, (dy, dx) in enumerate(TAPS):
                    if dx == 0:
                        src = xp[:, 1 + hr0 + dy : 1 + hr0 + dy + HR, XOFF : XOFF + W]
                    elif dx == 1:
                        src = xb13[:, 1 + hr0 + dy : 1 + hr0 + dy + HR, XOFF : XOFF + W]
                    else:
                        src = xb13[:, 1 + hr0 + dy : 1 + hr0 + dy + HR, 0:W]
                    if ti == 0:
                        nc.vector.tensor_scalar_mul(
                            out=sacc, in0=src, scalar1=ktab[:, ti : ti + 1]
                        )
                    else:
                        nc.vector.scalar_tensor_tensor(
                            out=sacc, in0=src, scalar=ktab[:, ti : ti + 1],
                            in1=sacc, op0=ALU.mult, op1=ALU.add,
                        )
                sl_h = slice(hr0 * W, (hr0 + HR) * W)
                nc.scalar.activation(srelu[:, sl_h], sacc, AF.Relu)
                for bb in range(BPH):
                    r0 = hr0 + bb * BR
                    tails.append(_make_tail(nc, pso_pool, opool, xp, xpl, srelu,
                                            None, cwt, attd, attv, cb, out_d,
                                            ci, r0, y0))
                    flush(TAIL_LAG)
            else:
                for bb in range(BPH):
                    r0 = hr0 + bb * BR
                    pss = pss_pool.tile([128, BR * W], F32, name=f"pss{u}_{bb}",
                                        tag="pss")
                    for ti, (dy, dx) in enumerate(TAPS):
                        rhs = xp[:, 1 + r0 + dy : 1 + r0 + dy + BR,
                                 XOFF + dx : XOFF + dx + W]
                        nc.tensor.matmul(
                            pss, lhsT=diags[ti], rhs=rhs,
                            start=(ti == 0), stop=(ti == NT - 1),
                        )
                    tails.append(_make_tail(nc, pso_pool, opool, xp, xpl, srelu,
                                            pss, cwt, attd, attv, cb, out_d,
                                            ci, r0, y0))
                    flush(TAIL_LAG)
    flush(0)


def _make_tail(nc, pso_pool, opool, xp, xpl, srelu, pss, cwt, attd, attv, cb,
               out_d, ci, r0, y0):
    """relu (PE halves) + pointwise + lo-residual + biased evac + hi-residual +
    store for the block at chunk-relative rows [r0, r0+BR)."""

    def tail():
        sl = slice(r0 * W, (r0 + BR) * W)
        if pss is not None:
            nc.scalar.activation(srelu[:, sl], pss, AF.Relu)
        pso = pso_pool.tile([128, BR * W], F32, name=f"pso{ci}_{r0}", tag="pso")
        nc.tensor.matmul(pso, lhsT=cwt, rhs=srelu[:, sl], start=True, stop=True)
        osb = opool.tile([128, BR * W], F32, name=f"ob{ci}_{r0}", tag="ob")
        nc.scalar.activation(osb, pso, AF.Identity, bias=cb)
        nc.vector.scalar_tensor_tensor(
            out=osb, in0=xp[:, 1 + r0 : 1 + r0 + BR, XOFF : XOFF + W],
            scalar=attv, in1=osb, op0=ALU.mult, op1=ALU.add,
        )
        nc.vector.scalar_tensor_tensor(
            out=osb, in0=xpl[:, 1 + r0 : 1 + r0 + BR, XOFF : XOFF + W],
            scalar=attv, in1=osb, op0=ALU.mult, op1=ALU.add,
        )
        nc.sync.dma_start(out=out_d[:, (y0 + r0) * W : (y0 + r0 + BR) * W], in_=osb)

    return tail


def build_module():
    nc = bacc.Bacc(
        "TRN2",
        target_bir_lowering=False,
        debug=False,
        enable_asserts=False,
        num_devices=B,
    )
    d = {
        "xpad_hi": nc.dram_tensor("xpad_hi", [C, HP * WP], BF16, kind="ExternalInput").ap(),
        "xpad_lo": nc.dram_tensor("xpad_lo", [C, HP * WP], BF16, kind="ExternalInput").ap(),
        "alt": nc.dram_tensor("alt", [128, 1], F32, kind="ExternalInput").ap(),
        "w1t": nc.dram_tensor("w1t", [128, 128], F32, kind="ExternalInput").ap(),
        "w2t": nc.dram_tensor("w2t", [128, C * NT], F32, kind="ExternalInput").ap(),
        "cwt": nc.dram_tensor("cwt", [C, C], BF16, kind="ExternalInput").ap(),
        "cb": nc.dram_tensor("cb", [C, 1], F32, kind="ExternalInput").ap(),
        "ca1t": nc.dram_tensor("ca1t", [128, 16], F32, kind="ExternalInput").ap(),
        "ca2t": nc.dram_tensor("ca2t", [16, 128], F32, kind="ExternalInput").ap(),
        "iota": nc.dram_tensor("iota", [128, 128], F32, kind="ExternalInput").ap(),
        "cidx": nc.dram_tensor("cidx", [128, 1], F32, kind="ExternalInput").ap(),
        "out": nc.dram_tensor("out", [C, HW], F32, kind="ExternalOutput").ap(),
    }
    with tile.TileContext(nc) as tc:
        with ExitStack() as ctx:
            d["ctx"] = ctx
            _emit(tc, nc, d)
    nc.finalize()
    return nc


_module_cache = None


def _get_module():
    global _module_cache
    if _module_cache is None:
        _module_cache = build_module()
    return _module_cache


def make_in_maps(x, altitude, W1, W2, conv_w, conv_b, ca_w1, ca_w2):
    f = np.float32
    x = np.asarray(x, dtype=f)
    altitude = np.asarray(altitude, dtype=f)
    xpad = np.zeros((B, C, HP, WP), dtype=f)
    xpad[:, :, 1 : H + 1, XOFF : XOFF + W] = x
    xhi_f = xpad.astype(ml_dtypes.bfloat16)
    xlo = np.ascontiguousarray(
        (xpad - xhi_f.astype(f)).astype(ml_dtypes.bfloat16).reshape(B, C, HP * WP)
    )
    xhi = np.ascontiguousarray(xhi_f.reshape(B, C, HP * WP))
    shared = {
        "w1t": np.ascontiguousarray(np.asarray(W1, dtype=f).T),
        "w2t": np.ascontiguousarray(np.asarray(W2, dtype=f).T),
        "cwt": np.ascontiguousarray(
            np.asarray(conv_w, dtype=f).T.astype(ml_dtypes.bfloat16)
        ),
        "cb": np.ascontiguousarray(np.asarray(conv_b, dtype=f).reshape(C, 1)),
        "ca1t": np.ascontiguousarray(np.asarray(ca_w1, dtype=f).T),
        "ca2t": np.ascontiguousarray(np.asarray(ca_w2, dtype=f).T),
        "iota": np.ascontiguousarray(
            np.broadcast_to(np.arange(128, dtype=f), (128, 128))
        ),
        "cidx": np.arange(128, dtype=f).reshape(128, 1).copy(),
    }
    return [
        dict(shared, xpad_hi=xhi[bb], xpad_lo=xlo[bb],
             alt=np.ascontiguousarray(altitude[bb].reshape(128, 1)))
        for bb in range(B)
    ]


def kernel(x, altitude, W1, W2, conv_w, conv_b, ca_w1, ca_w2):
    global last_results
    in_maps = make_in_maps(x, altitude, W1, W2, conv_w, conv_b, ca_w1, ca_w2)
    nc = _get_module()
    trace = os.environ.get("KERNEL_TRACE", "0") == "1"
    last_results = run_bass_kernel_spmd(
        nc, in_maps, core_ids=list(range(B)), trace=trace
    )
    out = np.stack(
        [last_results.results[bb]["out"].reshape(C, H, W) for bb in range(B)]
    )
    return out
